# revision 7
# baseline (speedup 1.0000x reference)
"""LocalizationAttacks kernel for 8 Trainium2 NeuronCores.

Data-parallel over the batch dim: each of the 8 cores processes 4 of the 32
batch items. The per-segment attack decisions (tiny [B, 300] masks) are
precomputed on the host from seg_starts/revert_flags and shipped to the device
as per-partition scalars; the 300 MB of audio streaming (2 input streams,
3 output streams) runs on-device, DMA-bound.

Per core the audio is viewed as [600 rows, 3200 f32] (2 segments of 1600
samples per SBUF partition row). For each [128, 3200] tile:
  attacked = wm * (1-am) + og * rm     (DVE: tensor_scalar_mul + fused stt)
  update_o = og * (1-zm)               (ACT: copy with per-partition scale)
  ground_t = broadcast(1-am)           (ACT: Relu(wm*0 + bias))
with per-partition [P,1] mask scalars, one column per packed segment.
"""

import numpy as np

import concourse.bacc as bacc
import concourse.bass as bass
import concourse.mybir as mybir
from concourse.bass_utils import run_bass_kernel_spmd
from concourse.tile import TileContext

# Problem shape (hardcoded per contract)
B, C, T = 32, 1, 480000
SEG = 1600
S = T // SEG              # 300 segments per item
N_CORES = 8
B_LOC = B // N_CORES      # 4 items per core
K = 2                     # segments packed per SBUF partition row
ROW = K * SEG             # 3200 f32 per row
N_ROWS = (B_LOC * S) // K  # 600 rows per core
P = 128

F32 = mybir.dt.float32


def _build_nc() -> bass.Bass:
    nc = bacc.Bacc()
    wm = nc.dram_tensor("wm", [N_ROWS, ROW], F32, kind="ExternalInput")
    og = nc.dram_tensor("og", [N_ROWS, ROW], F32, kind="ExternalInput")
    mk = nc.dram_tensor("mk", [N_ROWS, 3 * K], F32, kind="ExternalInput")
    att = nc.dram_tensor("att", [N_ROWS, ROW], F32, kind="ExternalOutput")
    gt = nc.dram_tensor("gt", [N_ROWS, ROW], F32, kind="ExternalOutput")
    uo = nc.dram_tensor("uo", [N_ROWS, ROW], F32, kind="ExternalOutput")

    mult = mybir.AluOpType.mult
    add = mybir.AluOpType.add

    with TileContext(nc) as tc:
        with tc.tile_pool(name="io", bufs=2) as pool:
            r0 = 0
            while r0 < N_ROWS:
                p = min(P, N_ROWS - r0)
                wm_t = pool.tile([p, ROW], F32, tag="wm", bufs=3)
                og_t = pool.tile([p, ROW], F32, tag="og", bufs=3)
                at_t = pool.tile([p, ROW], F32, tag="at")
                gt_t = pool.tile([p, ROW], F32, tag="gt")
                uo_t = pool.tile([p, ROW], F32, tag="uo")
                m_t = pool.tile([p, 3 * K], F32, tag="m", bufs=3)
                # Three DMA rings: audio loads on the SP HWDGE ring, att/uo
                # stores on the ACT HWDGE ring, mask loads + gt stores on the
                # GpSimd SWDGE ring. Each ring saturates ~280 GB/s on its own;
                # spreading keeps the aggregate at the HBM ceiling and keeps
                # the tiny mask DMAs off the big-load FIFO.
                nc.gpsimd.dma_start(out=m_t[:], in_=mk[r0 : r0 + p, :])
                nc.sync.dma_start(out=wm_t[:], in_=wm[r0 : r0 + p, :])
                nc.sync.dma_start(out=og_t[:], in_=og[r0 : r0 + p, :])
                for j in range(K):
                    sl = slice(j * SEG, (j + 1) * SEG)
                    s_am = m_t[:, j : j + 1]                   # 1 - attack
                    s_rm = m_t[:, K + j : K + j + 1]           # revert
                    s_zm = m_t[:, 2 * K + j : 2 * K + j + 1]   # 1 - zero
                    nc.vector.tensor_scalar_mul(at_t[:, sl], og_t[:, sl], s_rm)
                    nc.vector.scalar_tensor_tensor(
                        at_t[:, sl], wm_t[:, sl], s_am, at_t[:, sl], mult, add
                    )
                    nc.scalar.mul(uo_t[:, sl], og_t[:, sl], s_zm)
                    # broadcast of the per-partition mask: Relu(in*0 + bias)
                    nc.scalar.activation(
                        gt_t[:, sl],
                        wm_t[:, sl],
                        mybir.ActivationFunctionType.Relu,
                        bias=s_am,
                        scale=0.0,
                    )
                nc.scalar.dma_start(out=att[r0 : r0 + p, :], in_=at_t[:])
                nc.gpsimd.dma_start(out=gt[r0 : r0 + p, :], in_=gt_t[:])
                nc.scalar.dma_start(out=uo[r0 : r0 + p, :], in_=uo_t[:])
                r0 += p
    nc.compile()
    return nc


_NC_CACHE: bass.Bass | None = None


def _prepare_in_maps(original, watermarked, seg_starts, revert_flags):
    original = np.ascontiguousarray(np.asarray(original), dtype=np.float32)
    watermarked = np.ascontiguousarray(np.asarray(watermarked), dtype=np.float32)
    seg_starts = np.asarray(seg_starts)
    revert_flags = np.asarray(revert_flags)

    # Host-side segment masks, [B, 300] each (tiny).
    attack = np.zeros((B, S), np.float32)
    attack[np.arange(B)[:, None], seg_starts] = 1.0
    rf = revert_flags.astype(np.float32)
    one_minus_am = 1.0 - attack
    rm = attack * rf
    one_minus_zm = 1.0 - attack * (1.0 - rf)

    in_maps = []
    for c in range(N_CORES):
        sl = slice(c * B_LOC, (c + 1) * B_LOC)
        mk = np.concatenate(
            [
                one_minus_am[sl].reshape(N_ROWS, K),
                rm[sl].reshape(N_ROWS, K),
                one_minus_zm[sl].reshape(N_ROWS, K),
            ],
            axis=1,
        )
        in_maps.append(
            {
                "wm": watermarked[sl].reshape(N_ROWS, ROW),
                "og": original[sl].reshape(N_ROWS, ROW),
                "mk": np.ascontiguousarray(mk),
            }
        )
    return in_maps


def _gather(results):
    def cat(name):
        return np.concatenate(
            [results[c][name].reshape(B_LOC, C, T) for c in range(N_CORES)], axis=0
        )

    return cat("att"), cat("gt"), cat("uo")


def _run(inputs: dict, **run_kwargs):
    global _NC_CACHE
    if _NC_CACHE is None:
        _NC_CACHE = _build_nc()
    in_maps = _prepare_in_maps(**inputs)
    res = run_bass_kernel_spmd(
        _NC_CACHE, in_maps, core_ids=list(range(N_CORES)), **run_kwargs
    )
    return res, _gather(res.results)


def kernel(original, watermarked, seg_starts, revert_flags):
    _, outs = _run(
        dict(
            original=original,
            watermarked=watermarked,
            seg_starts=seg_starts,
            revert_flags=revert_flags,
        )
    )
    return outs


# revision 10
# speedup vs baseline: 1.0050x; 1.0050x over previous
"""LocalizationAttacks kernel for 8 Trainium2 NeuronCores.

Data-parallel over the batch dim: each of the 8 cores processes 4 of the 32
batch items. The per-segment attack decisions (tiny [B, 300] masks) are
precomputed on the host from seg_starts/revert_flags and shipped to the device
as per-partition scalars; the 300 MB of audio streaming (2 input streams,
3 output streams) runs on-device, DMA-bound.

Per core the audio is viewed as [600 rows, 3200 f32] (2 segments of 1600
samples per SBUF partition row). For each [128, 3200] tile:
  attacked = wm * (1-am) + og * rm     (DVE: tensor_scalar_mul + fused stt)
  update_o = og * (1-zm)               (ACT: copy with per-partition scale)
  ground_t = broadcast(1-am)           (ACT: Relu(wm*0 + bias))
with per-partition [P,1] mask scalars, one column per packed segment.
"""

import numpy as np

import concourse.bacc as bacc
import concourse.bass as bass
import concourse.mybir as mybir
from concourse.bass_utils import run_bass_kernel_spmd
from concourse.tile import TileContext

# Problem shape (hardcoded per contract)
B, C, T = 32, 1, 480000
SEG = 1600
S = T // SEG              # 300 segments per item
N_CORES = 8
B_LOC = B // N_CORES      # 4 items per core
K = 2                     # segments packed per SBUF partition row
ROW = K * SEG             # 3200 f32 per row
N_ROWS = (B_LOC * S) // K  # 600 rows per core
P = 128

F32 = mybir.dt.float32


def _build_nc() -> bass.Bass:
    nc = bacc.Bacc()
    wm = nc.dram_tensor("wm", [N_ROWS, ROW], F32, kind="ExternalInput")
    og = nc.dram_tensor("og", [N_ROWS, ROW], F32, kind="ExternalInput")
    mk = nc.dram_tensor("mk", [N_ROWS, 3 * K], F32, kind="ExternalInput")
    att = nc.dram_tensor("att", [N_ROWS, ROW], F32, kind="ExternalOutput")
    gt = nc.dram_tensor("gt", [N_ROWS, ROW], F32, kind="ExternalOutput")
    uo = nc.dram_tensor("uo", [N_ROWS, ROW], F32, kind="ExternalOutput")

    mult = mybir.AluOpType.mult
    add = mybir.AluOpType.add

    n_iters = (N_ROWS + P - 1) // P
    with TileContext(nc) as tc:
        with tc.tile_pool(name="io", bufs=2) as pool:
            deferred = []  # (dram_ap, tile) stores moved to the sync ring
            r0 = 0
            it = 0
            while r0 < N_ROWS:
                p = min(P, N_ROWS - r0)
                wm_t = pool.tile([p, ROW], F32, tag="wm", bufs=3)
                og_t = pool.tile([p, ROW], F32, tag="og", bufs=3)
                at_t = pool.tile([p, ROW], F32, tag="at")
                gt_t = pool.tile([p, ROW], F32, tag="gt")
                uo_t = pool.tile([p, ROW], F32, tag="uo")
                m_t = pool.tile([p, 3 * K], F32, tag="m", bufs=3)
                # Two HWDGE rings: loads on SP, stores on ACT. Dual-active
                # aggregate reaches ~430 GB/s (fabric-limited); a lone ring
                # sags, so the byte split is balanced by deferring the last
                # ~4 MB of stores to the SP ring after its loads are done.
                nc.sync.dma_start(out=m_t[:], in_=mk[r0 : r0 + p, :])
                nc.sync.dma_start(out=wm_t[:], in_=wm[r0 : r0 + p, :])
                nc.sync.dma_start(out=og_t[:], in_=og[r0 : r0 + p, :])
                for j in range(K):
                    sl = slice(j * SEG, (j + 1) * SEG)
                    s_am = m_t[:, j : j + 1]                   # 1 - attack
                    s_rm = m_t[:, K + j : K + j + 1]           # revert
                    s_zm = m_t[:, 2 * K + j : 2 * K + j + 1]   # 1 - zero
                    nc.vector.tensor_scalar_mul(at_t[:, sl], og_t[:, sl], s_rm)
                    nc.vector.scalar_tensor_tensor(
                        at_t[:, sl], wm_t[:, sl], s_am, at_t[:, sl], mult, add
                    )
                    nc.scalar.mul(uo_t[:, sl], og_t[:, sl], s_zm)
                    # broadcast of the per-partition mask: Relu(in*0 + bias)
                    nc.scalar.activation(
                        gt_t[:, sl],
                        wm_t[:, sl],
                        mybir.ActivationFunctionType.Relu,
                        bias=s_am,
                        scale=0.0,
                    )
                nc.scalar.dma_start(out=att[r0 : r0 + p, :], in_=at_t[:])
                if it >= n_iters - 2:
                    deferred.append((gt[r0 : r0 + p, :], gt_t))
                else:
                    nc.scalar.dma_start(out=gt[r0 : r0 + p, :], in_=gt_t[:])
                if it >= n_iters - 1:
                    deferred.append((uo[r0 : r0 + p, :], uo_t))
                else:
                    nc.scalar.dma_start(out=uo[r0 : r0 + p, :], in_=uo_t[:])
                r0 += p
                it += 1
            for dram_ap, t in deferred:
                nc.sync.dma_start(out=dram_ap, in_=t[:])
    nc.compile()
    return nc


_NC_CACHE: bass.Bass | None = None


def _prepare_in_maps(original, watermarked, seg_starts, revert_flags):
    original = np.ascontiguousarray(np.asarray(original), dtype=np.float32)
    watermarked = np.ascontiguousarray(np.asarray(watermarked), dtype=np.float32)
    seg_starts = np.asarray(seg_starts)
    revert_flags = np.asarray(revert_flags)

    # Host-side segment masks, [B, 300] each (tiny).
    attack = np.zeros((B, S), np.float32)
    attack[np.arange(B)[:, None], seg_starts] = 1.0
    rf = revert_flags.astype(np.float32)
    one_minus_am = 1.0 - attack
    rm = attack * rf
    one_minus_zm = 1.0 - attack * (1.0 - rf)

    in_maps = []
    for c in range(N_CORES):
        sl = slice(c * B_LOC, (c + 1) * B_LOC)
        mk = np.concatenate(
            [
                one_minus_am[sl].reshape(N_ROWS, K),
                rm[sl].reshape(N_ROWS, K),
                one_minus_zm[sl].reshape(N_ROWS, K),
            ],
            axis=1,
        )
        in_maps.append(
            {
                "wm": watermarked[sl].reshape(N_ROWS, ROW),
                "og": original[sl].reshape(N_ROWS, ROW),
                "mk": np.ascontiguousarray(mk),
            }
        )
    return in_maps


def _gather(results):
    def cat(name):
        return np.concatenate(
            [results[c][name].reshape(B_LOC, C, T) for c in range(N_CORES)], axis=0
        )

    return cat("att"), cat("gt"), cat("uo")


def _run(inputs: dict, **run_kwargs):
    global _NC_CACHE
    if _NC_CACHE is None:
        _NC_CACHE = _build_nc()
    in_maps = _prepare_in_maps(**inputs)
    res = run_bass_kernel_spmd(
        _NC_CACHE, in_maps, core_ids=list(range(N_CORES)), **run_kwargs
    )
    return res, _gather(res.results)


def kernel(original, watermarked, seg_starts, revert_flags):
    _, outs = _run(
        dict(
            original=original,
            watermarked=watermarked,
            seg_starts=seg_starts,
            revert_flags=revert_flags,
        )
    )
    return outs


# revision 11
# speedup vs baseline: 1.0511x; 1.0459x over previous
"""LocalizationAttacks kernel for 8 Trainium2 NeuronCores.

Data-parallel over the batch dim: each of the 8 cores processes 4 of the 32
batch items. The per-segment attack decisions (tiny [B, 300] masks) are
precomputed on the host from seg_starts/revert_flags and shipped to the device
as per-partition scalars; the 300 MB of audio streaming (2 input streams,
3 output streams) runs on-device, DMA-bound.

Per core the audio is viewed as [600 rows, 3200 f32] (2 segments of 1600
samples per SBUF partition row). For each [128, 3200] tile:
  attacked = wm * (1-am) + og * rm     (DVE: tensor_scalar_mul + fused stt)
  update_o = og * (1-zm)               (ACT: copy with per-partition scale)
  ground_t = broadcast(1-am)           (ACT: Relu(wm*0 + bias))
with per-partition [P,1] mask scalars, one column per packed segment.
"""

import numpy as np

import concourse.bacc as bacc
import concourse.bass as bass
import concourse.mybir as mybir
from concourse.bass_utils import run_bass_kernel_spmd
from concourse.tile import TileContext

# Problem shape (hardcoded per contract)
B, C, T = 32, 1, 480000
SEG = 1600
S = T // SEG              # 300 segments per item
N_CORES = 8
B_LOC = B // N_CORES      # 4 items per core
K = 2                     # segments packed per SBUF partition row
ROW = K * SEG             # 3200 f32 per row
N_ROWS = (B_LOC * S) // K  # 600 rows per core
P = 128

F32 = mybir.dt.float32


def _build_nc() -> bass.Bass:
    nc = bacc.Bacc()
    wm = nc.dram_tensor("wm", [N_ROWS, ROW], F32, kind="ExternalInput")
    og = nc.dram_tensor("og", [N_ROWS, ROW], F32, kind="ExternalInput")
    mk = nc.dram_tensor("mk", [N_ROWS, 3 * K], F32, kind="ExternalInput")
    att = nc.dram_tensor("att", [N_ROWS, ROW], F32, kind="ExternalOutput")
    gt = nc.dram_tensor("gt", [N_ROWS, ROW], F32, kind="ExternalOutput")
    uo = nc.dram_tensor("uo", [N_ROWS, ROW], F32, kind="ExternalOutput")

    mult = mybir.AluOpType.mult
    add = mybir.AluOpType.add

    n_iters = (N_ROWS + P - 1) // P
    with TileContext(nc) as tc:
        with tc.tile_pool(name="io", bufs=2) as pool:
            deferred = []  # (dram_ap, tile) stores moved to the sync ring
            r0 = 0
            it = 0
            while r0 < N_ROWS:
                p = min(P, N_ROWS - r0)
                wm_t = pool.tile([p, ROW], F32, tag="wm", bufs=3)
                og_t = pool.tile([p, ROW], F32, tag="og", bufs=3)
                at_t = pool.tile([p, ROW], F32, tag="at")
                gt_t = pool.tile([p, ROW], F32, tag="gt")
                uo_t = pool.tile([p, ROW], F32, tag="uo")
                m_t = pool.tile([p, 3 * K], F32, tag="m", bufs=3)
                # Two HWDGE rings: loads on SP, stores on ACT. Dual-active
                # aggregate reaches ~430 GB/s (fabric-limited); a lone ring
                # sags, so the byte split is balanced by deferring the last
                # ~4 MB of stores to the SP ring after its loads are done.
                nc.sync.dma_start(out=m_t[:], in_=mk[r0 : r0 + p, :])
                nc.sync.dma_start(out=wm_t[:], in_=wm[r0 : r0 + p, :])
                nc.sync.dma_start(out=og_t[:], in_=og[r0 : r0 + p, :])
                for j in range(K):
                    sl = slice(j * SEG, (j + 1) * SEG)
                    s_am = m_t[:, j : j + 1]                   # 1 - attack
                    s_rm = m_t[:, K + j : K + j + 1]           # revert
                    s_zm = m_t[:, 2 * K + j : 2 * K + j + 1]   # 1 - zero
                    nc.vector.tensor_scalar_mul(at_t[:, sl], og_t[:, sl], s_rm)
                    nc.vector.scalar_tensor_tensor(
                        at_t[:, sl], wm_t[:, sl], s_am, at_t[:, sl], mult, add
                    )
                    nc.scalar.mul(uo_t[:, sl], og_t[:, sl], s_zm)
                    # broadcast of the per-partition mask: Relu(in*0 + bias)
                    nc.scalar.activation(
                        gt_t[:, sl],
                        wm_t[:, sl],
                        mybir.ActivationFunctionType.Relu,
                        bias=s_am,
                        scale=0.0,
                    )
                nc.scalar.dma_start(out=att[r0 : r0 + p, :], in_=at_t[:])
                # balance ring bytes: send gt of odd iterations down the SP
                # ring (issued in-loop so no tile-slot pinning)
                gt_ring = nc.sync if it % 2 == 1 else nc.scalar
                gt_ring.dma_start(out=gt[r0 : r0 + p, :], in_=gt_t[:])
                nc.scalar.dma_start(out=uo[r0 : r0 + p, :], in_=uo_t[:])
                r0 += p
                it += 1
    nc.compile()
    return nc


_NC_CACHE: bass.Bass | None = None


def _prepare_in_maps(original, watermarked, seg_starts, revert_flags):
    original = np.ascontiguousarray(np.asarray(original), dtype=np.float32)
    watermarked = np.ascontiguousarray(np.asarray(watermarked), dtype=np.float32)
    seg_starts = np.asarray(seg_starts)
    revert_flags = np.asarray(revert_flags)

    # Host-side segment masks, [B, 300] each (tiny).
    attack = np.zeros((B, S), np.float32)
    attack[np.arange(B)[:, None], seg_starts] = 1.0
    rf = revert_flags.astype(np.float32)
    one_minus_am = 1.0 - attack
    rm = attack * rf
    one_minus_zm = 1.0 - attack * (1.0 - rf)

    in_maps = []
    for c in range(N_CORES):
        sl = slice(c * B_LOC, (c + 1) * B_LOC)
        mk = np.concatenate(
            [
                one_minus_am[sl].reshape(N_ROWS, K),
                rm[sl].reshape(N_ROWS, K),
                one_minus_zm[sl].reshape(N_ROWS, K),
            ],
            axis=1,
        )
        in_maps.append(
            {
                "wm": watermarked[sl].reshape(N_ROWS, ROW),
                "og": original[sl].reshape(N_ROWS, ROW),
                "mk": np.ascontiguousarray(mk),
            }
        )
    return in_maps


def _gather(results):
    def cat(name):
        return np.concatenate(
            [results[c][name].reshape(B_LOC, C, T) for c in range(N_CORES)], axis=0
        )

    return cat("att"), cat("gt"), cat("uo")


def _run(inputs: dict, **run_kwargs):
    global _NC_CACHE
    if _NC_CACHE is None:
        _NC_CACHE = _build_nc()
    in_maps = _prepare_in_maps(**inputs)
    res = run_bass_kernel_spmd(
        _NC_CACHE, in_maps, core_ids=list(range(N_CORES)), **run_kwargs
    )
    return res, _gather(res.results)


def kernel(original, watermarked, seg_starts, revert_flags):
    _, outs = _run(
        dict(
            original=original,
            watermarked=watermarked,
            seg_starts=seg_starts,
            revert_flags=revert_flags,
        )
    )
    return outs


# revision 12
# speedup vs baseline: 1.0589x; 1.0074x over previous
"""LocalizationAttacks kernel for 8 Trainium2 NeuronCores.

Data-parallel over the batch dim: each of the 8 cores processes 4 of the 32
batch items. The per-segment attack decisions (tiny [B, 300] masks) are
precomputed on the host from seg_starts/revert_flags and shipped to the device
as per-partition scalars; the 300 MB of audio streaming (2 input streams,
3 output streams) runs on-device, DMA-bound.

Per core the audio is viewed as [600 rows, 3200 f32] (2 segments of 1600
samples per SBUF partition row). For each [128, 3200] tile:
  attacked = wm * (1-am) + og * rm     (DVE: tensor_scalar_mul + fused stt)
  update_o = og * (1-zm)               (ACT: copy with per-partition scale)
  ground_t = broadcast(1-am)           (ACT: Relu(wm*0 + bias))
with per-partition [P,1] mask scalars, one column per packed segment.
"""

import numpy as np

import concourse.bacc as bacc
import concourse.bass as bass
import concourse.mybir as mybir
from concourse.bass_utils import run_bass_kernel_spmd
from concourse.tile import TileContext

# Problem shape (hardcoded per contract)
B, C, T = 32, 1, 480000
SEG = 1600
S = T // SEG              # 300 segments per item
N_CORES = 8
B_LOC = B // N_CORES      # 4 items per core
K = 2                     # segments packed per SBUF partition row
ROW = K * SEG             # 3200 f32 per row
N_ROWS = (B_LOC * S) // K  # 600 rows per core
P = 128

F32 = mybir.dt.float32


def _build_nc() -> bass.Bass:
    nc = bacc.Bacc()
    wm = nc.dram_tensor("wm", [N_ROWS, ROW], F32, kind="ExternalInput")
    og = nc.dram_tensor("og", [N_ROWS, ROW], F32, kind="ExternalInput")
    mk = nc.dram_tensor("mk", [N_ROWS, 3 * K], F32, kind="ExternalInput")
    att = nc.dram_tensor("att", [N_ROWS, ROW], F32, kind="ExternalOutput")
    gt = nc.dram_tensor("gt", [N_ROWS, ROW], F32, kind="ExternalOutput")
    uo = nc.dram_tensor("uo", [N_ROWS, ROW], F32, kind="ExternalOutput")

    mult = mybir.AluOpType.mult
    add = mybir.AluOpType.add

    n_iters = (N_ROWS + P - 1) // P
    with TileContext(nc) as tc:
        with tc.tile_pool(name="io", bufs=2) as pool:
            deferred = []  # (dram_ap, tile) stores moved to the sync ring
            r0 = 0
            it = 0
            while r0 < N_ROWS:
                p = min(P, N_ROWS - r0)
                wm_t = pool.tile([p, ROW], F32, tag="wm", bufs=3)
                og_t = pool.tile([p, ROW], F32, tag="og", bufs=3)
                at_t = pool.tile([p, ROW], F32, tag="at")
                gt_t = pool.tile([p, ROW], F32, tag="gt")
                uo_t = pool.tile([p, ROW], F32, tag="uo")
                m_t = pool.tile([p, 3 * K], F32, tag="m", bufs=3)
                # Two HWDGE rings: loads on SP, stores on ACT. Dual-active
                # aggregate reaches ~430 GB/s (fabric-limited); a lone ring
                # sags, so the byte split is balanced by deferring the last
                # ~4 MB of stores to the SP ring after its loads are done.
                nc.sync.dma_start(out=m_t[:], in_=mk[r0 : r0 + p, :])
                nc.sync.dma_start(out=wm_t[:], in_=wm[r0 : r0 + p, :])
                nc.sync.dma_start(out=og_t[:], in_=og[r0 : r0 + p, :])
                for j in range(K):
                    sl = slice(j * SEG, (j + 1) * SEG)
                    s_am = m_t[:, j : j + 1]                   # 1 - attack
                    s_rm = m_t[:, K + j : K + j + 1]           # revert
                    s_zm = m_t[:, 2 * K + j : 2 * K + j + 1]   # 1 - zero
                    nc.vector.tensor_scalar_mul(at_t[:, sl], og_t[:, sl], s_rm)
                    nc.vector.scalar_tensor_tensor(
                        at_t[:, sl], wm_t[:, sl], s_am, at_t[:, sl], mult, add
                    )
                    nc.scalar.mul(uo_t[:, sl], og_t[:, sl], s_zm)
                    # broadcast of the per-partition mask: Relu(in*0 + bias)
                    nc.scalar.activation(
                        gt_t[:, sl],
                        wm_t[:, sl],
                        mybir.ActivationFunctionType.Relu,
                        bias=s_am,
                        scale=0.0,
                    )
                nc.scalar.dma_start(out=att[r0 : r0 + p, :], in_=at_t[:])
                nc.scalar.dma_start(out=gt[r0 : r0 + p, :], in_=gt_t[:])
                nc.scalar.dma_start(out=uo[r0 : r0 + p, :], in_=uo_t[:])
                r0 += p
                it += 1
    nc.compile()
    return nc


_NC_CACHE: bass.Bass | None = None


def _prepare_in_maps(original, watermarked, seg_starts, revert_flags):
    original = np.ascontiguousarray(np.asarray(original), dtype=np.float32)
    watermarked = np.ascontiguousarray(np.asarray(watermarked), dtype=np.float32)
    seg_starts = np.asarray(seg_starts)
    revert_flags = np.asarray(revert_flags)

    # Host-side segment masks, [B, 300] each (tiny).
    attack = np.zeros((B, S), np.float32)
    attack[np.arange(B)[:, None], seg_starts] = 1.0
    rf = revert_flags.astype(np.float32)
    one_minus_am = 1.0 - attack
    rm = attack * rf
    one_minus_zm = 1.0 - attack * (1.0 - rf)

    in_maps = []
    for c in range(N_CORES):
        sl = slice(c * B_LOC, (c + 1) * B_LOC)
        mk = np.concatenate(
            [
                one_minus_am[sl].reshape(N_ROWS, K),
                rm[sl].reshape(N_ROWS, K),
                one_minus_zm[sl].reshape(N_ROWS, K),
            ],
            axis=1,
        )
        in_maps.append(
            {
                "wm": watermarked[sl].reshape(N_ROWS, ROW),
                "og": original[sl].reshape(N_ROWS, ROW),
                "mk": np.ascontiguousarray(mk),
            }
        )
    return in_maps


def _gather(results):
    def cat(name):
        return np.concatenate(
            [results[c][name].reshape(B_LOC, C, T) for c in range(N_CORES)], axis=0
        )

    return cat("att"), cat("gt"), cat("uo")


def _run(inputs: dict, **run_kwargs):
    global _NC_CACHE
    if _NC_CACHE is None:
        _NC_CACHE = _build_nc()
    in_maps = _prepare_in_maps(**inputs)
    res = run_bass_kernel_spmd(
        _NC_CACHE, in_maps, core_ids=list(range(N_CORES)), **run_kwargs
    )
    return res, _gather(res.results)


def kernel(original, watermarked, seg_starts, revert_flags):
    _, outs = _run(
        dict(
            original=original,
            watermarked=watermarked,
            seg_starts=seg_starts,
            revert_flags=revert_flags,
        )
    )
    return outs
